# Initial kernel scaffold
#
"""Trainium2 Bass kernel for a dense transformer layer (B=4, T=2048, D=1024,
H=16, HD=64, FF=4096, fp32 I/O).

Sharding (8 cores, no cross-core communication): core c handles batch c//2 and
token-half c%2; per-core inputs are permuted so the core's own 1024 tokens come
first. K/V cover all 2048 tokens of the batch (2x redundant); Q/attention/Wo/MLP
cover only the core's 1024 tokens.

v1 redesign vs the token-major baseline:
- Feature-major activations end to end: x arrives transposed from the host, the
  residual stream stays feature-major, and the output is written transposed
  (host un-transposes). This removes all PE transposes (and enables walrus'
  redundant-LDWEIGHTS optimization, which is incompatible with transpose-mode
  weight loads).
- LayerNorm stats via ones-vector matmuls over feature chunks (PSUM row
  accumulation), normalization applied with gpsimd partition_broadcast'ed
  mean/rstd rows; LN gains/biases and all projection biases are folded into
  weights host-side, rank-1 corrections ride as K=1 matmuls.
- rstd = exp(-0.5*log(var+eps)) keeps ACT in the natural_log_exp table set
  (shared with attention's exp), avoiding Sqrt table switches.
- Rope partner-swap via 4 SBUF->SBUF DMAs per tile instead of 4 narrow DVE
  tensor_tensor ops; softmax denominators via reciprocal_approx_fast (~5x
  faster than reciprocal) broadcast on gpsimd.
"""

import sys

sys.path.insert(0, "/opt/trn_rl_repo")

import dataclasses

import numpy as np
import ml_dtypes

import concourse.bass as bass
import concourse.tile as tile
from concourse import library_config, mybir
from concourse.vector_clock import ScopedClock

F32 = mybir.dt.float32
BF16 = mybir.dt.bfloat16
AF = mybir.ActivationFunctionType
ALU = mybir.AluOpType

B, T, D = 4, 2048, 1024
H, HD = 16, 64
FF = 4 * D
MY = T // 2          # tokens owned by one core
KC = T // 128        # k chunks of 128 tokens
EPS = 1e-5
SCALE = 1.0 / 8.0    # 1/sqrt(HD)

BF = ml_dtypes.bfloat16


class PatchedTileContext(tile.TileContext):
    """walrus in this container accepts a single sync-wait per instruction;
    split the tail drain's waits across chained drains."""

    def _drain_and_barrier(self, tick_clock, wait_clock):
        drain_inst = self.nc.sync.drain()
        wait_clock.add_sem_waits(
            drain_inst.ins, ScopedClock({None: tick_clock.global_clock})
        )
        si = drain_inst.ins.sync_info
        waits = list(si.on_wait) if si and si.on_wait else []
        if len(waits) > 1:
            si.on_wait = waits[:1]
            for w in waits[1:]:
                d2 = self.nc.sync.drain()
                si2 = d2.ins.sync_info
                if si2 is None:
                    d2.ins.sync_info = mybir.SyncInfo(on_wait=[w], on_update=[])
                else:
                    si2.on_wait = [w]
        self.nc.all_engine_barrier()
        assert self.sems is not None
        popped = self.nc._tile_sem_poison_stack.pop()
        assert popped is self._sem_poison
        self.nc.clear_and_free_semaphores(list(self.sems.allocated().values()))
        self.nc.all_engine_barrier()


def split_multi_waits(nc, max_waits=1):
    """Move extra sync-waits onto NoOps inserted just before the over-limit
    instruction (same engine, program order preserved)."""
    template = nc.vector.nop().ins
    fn = nc.m.functions[0]
    ctr = 0
    for bb in fn.blocks:
        out = []
        for inst in bb.instructions:
            si = getattr(inst, "sync_info", None)
            waits = list(si.on_wait) if (si and si.on_wait) else []
            if len(waits) > max_waits:
                for w in waits[:-max_waits]:
                    ctr += 1
                    nop = dataclasses.replace(
                        template,
                        name=f"IWS-{ctr}",
                        engine=inst.engine,
                        ins=[],
                        outs=[],
                        sync_info=mybir.SyncInfo(on_wait=[w], on_update=[]),
                    )
                    nc.register_instruction(nop, overwrite=True)
                    out.append(nop)
                si.on_wait = waits[-max_waits:]
            out.append(inst)
        bb.instructions[:] = out
    return ctr


def dedupe_ldweights(nc):
    """Remove InstLdweights whose stationary operand is identical to the
    immediately preceding weight load, with only PE matmuls/noops in between
    (weights persist in the PE array across matmuls). Conservative: keeps
    any LDW carrying sem waits, and resets tracking whenever a non-PE
    instruction appears (it may rewrite the SBUF weight region)."""
    fn = nc.m.functions[0]
    removed = 0
    for bb in fn.blocks:
        out = []
        prev_key = None
        for inst in bb.instructions:
            if isinstance(inst, mybir.InstLdweights):
                ap = inst.ins[0]
                key = (ap.memref, ap.offset, str(ap.ap), str(ap.dtype),
                       getattr(inst, "is_transpose", None),
                       getattr(inst, "perf_mode", None),
                       str(getattr(inst, "tile_position", None)))
                si = inst.sync_info
                has_sync = bool(si and (si.on_wait or si.on_update))
                if key == prev_key and not has_sync:
                    removed += 1
                    continue
                prev_key = key
            elif isinstance(inst, (mybir.InstMatmult, mybir.InstNoOp)):
                pass
            else:
                prev_key = None
            out.append(inst)
        bb.instructions[:] = out
    return removed


def build_program(has_vbias=True):
    nc = bass.Bass()

    # ---- I/O (all weights host-folded: W' = ln_g ⊙ W; biases separate) ----
    xT_bf = nc.declare_dram_parameter("xT_bf", [D, T], BF16, isOutput=False)
    xT_own = nc.declare_dram_parameter("xT_own", [D, MY], F32, isOutput=False)
    wqkv = nc.declare_dram_parameter("wqkv", [D, 3 * D], BF16, isOutput=False)
    wo = nc.declare_dram_parameter("wo", [D, D], BF16, isOutput=False)
    w1 = nc.declare_dram_parameter("w1", [D, FF], BF16, isOutput=False)
    # w2p[do*128+p, fc*128+j] = W2[fc*128+p, do*128+j]
    w2p = nc.declare_dram_parameter("w2p", [D, FF], BF16, isOutput=False)
    cos_in = nc.declare_dram_parameter("cos_rep", [128, T], BF16, isOutput=False)
    sinsw_in = nc.declare_dram_parameter("sinsw_rep", [128, T], BF16, isOutput=False)
    bqc_in = nc.declare_dram_parameter("bqc", [128, 16], F32, isOutput=False)  # q/k bias cols
    bqv_in = nc.declare_dram_parameter("bqv", [1, D], BF16, isOutput=False)    # v bias row
    b1c_in = nc.declare_dram_parameter("b1c", [128, FF // 128], F32, isOutput=False)
    boc_in = nc.declare_dram_parameter("boc", [128, 8], F32, isOutput=False)
    b2c_in = nc.declare_dram_parameter("b2c", [128, 8], F32, isOutput=False)
    outT = nc.declare_dram_parameter("outT", [D, MY], F32, isOutput=True)

    with PatchedTileContext(nc) as tc:
        consts_cm = tc.tile_pool(name="consts", bufs=1)
        consts = consts_cm.__enter__()
        ones_bf = consts.tile([128, 128], BF16)
        nc.vector.memset(ones_bf[:], 1.0)
        eps_col = consts.tile([128, 1], F32)
        nc.vector.memset(eps_col[:], EPS)
        onesrow = consts.tile([1, T], BF16)
        nc.vector.memset(onesrow[:], 1.0)
        ones64 = consts.tile([1, 64], BF16)
        nc.vector.memset(ones64[:], 1.0)
        bqc = consts.tile([128, 16], F32)
        nc.gpsimd.dma_start(out=bqc[:], in_=bqc_in[:])
        bqv = consts.tile([1, D], BF16)
        nc.gpsimd.dma_start(out=bqv[:], in_=bqv_in[:])
        b1c = consts.tile([128, FF // 128], F32)
        nc.gpsimd.dma_start(out=b1c[:], in_=b1c_in[:])
        boc = consts.tile([128, 8], F32)
        nc.gpsimd.dma_start(out=boc[:], in_=boc_in[:])
        b2c = consts.tile([128, 8], F32)
        nc.gpsimd.dma_start(out=b2c[:], in_=b2c_in[:])

        cossin_cm = tc.tile_pool(name="cossin", bufs=1)
        cossin = cossin_cm.__enter__()
        cos_sb = cossin.tile([128, T], BF16)
        sinsw_sb = cossin.tile([128, T], BF16)

        # x_hat pool opened early (left stack: outlives wqk/wv/ln1)
        xh_cm = tc.tile_pool(name="xh", bufs=8)
        xhp = xh_cm.__enter__()
        xh = [xhp.tile([128, T], BF16, tag="xh", name=f"xh{i}") for i in range(8)]

        # v-half of wqkv: loaded during q/k projections, dies after v
        wv_cm = tc.tile_pool(name="wv", bufs=8)
        wvp = wv_cm.__enter__()
        wv_sb = [wvp.tile([128, D], BF16, tag="wv", name=f"wv{i}") for i in range(8)]

        wqk_cm = tc.tile_pool(name="wqk", bufs=8)
        wqkp = wqk_cm.__enter__()
        wqk_sb = [wqkp.tile([128, 2 * D], BF16, tag="wqk", name=f"wqk{i}") for i in range(8)]

        # ---------- S1a: load xT (right stack), stats via ones-matmuls ----------
        xTp_cm = tc.tile_pool(name="xT", bufs=8, side="right")
        xTp = xTp_cm.__enter__()
        xT = [xTp.tile([128, T], BF16, tag="xT", name=f"xT{i}") for i in range(8)]
        for dn in range(8):
            nc.gpsimd.dma_start(out=xT[dn][:], in_=xT_bf[dn * 128:(dn + 1) * 128, :])
        nc.gpsimd.dma_start(out=cos_sb[:], in_=cos_in[:])
        nc.gpsimd.dma_start(out=sinsw_sb[:], in_=sinsw_in[:])
        for dn in range(8):
            nc.gpsimd.dma_start(out=wqk_sb[dn][:], in_=wqkv[dn * 128:(dn + 1) * 128, 0:2 * D])

        ln1_cm = tc.tile_pool(name="ln1", bufs=1)
        ln1 = ln1_cm.__enter__()
        A_rep = ln1.tile([128, T], BF16)      # rstd broadcast
        nmu_rep = ln1.tile([128, T], BF16)    # -mu broadcast

        with tc.tile_pool(name="s1stat", bufs=2) as pst, \
             tc.tile_pool(name="s1stat_ps", bufs=1, space="PSUM") as pst_ps:
            s1ps = pst_ps.tile([128, T], F32, tag="s1ps", name="s1ps")
            s2ps = pst_ps.tile([128, T], F32, tag="s2ps", name="s2ps")
            for dn in range(8):
                x2 = pst.tile([128, T], BF16, tag="x2")
                nc.scalar.activation(out=x2[:], in_=xT[dn][:], func=AF.Square)
                for j in range(4):
                    nc.tensor.matmul(
                        s1ps[:, j * 512:(j + 1) * 512], ones_bf[:],
                        xT[dn][:, j * 512:(j + 1) * 512],
                        start=(dn == 0), stop=(dn == 7))
                    nc.tensor.matmul(
                        s2ps[:, j * 512:(j + 1) * 512], ones_bf[:],
                        x2[:, j * 512:(j + 1) * 512],
                        start=(dn == 0), stop=(dn == 7))
            c = 1.0 / D
            mu = pst.tile([128, T], F32, tag="sc", name="mu")
            nc.vector.tensor_scalar_mul(mu[:], s1ps[:], c)
            mu2 = pst.tile([128, T], F32, tag="sc", name="mu2")
            nc.vector.tensor_tensor(out=mu2[:], in0=mu[:], in1=mu[:], op=ALU.mult)
            var = pst.tile([128, T], F32, tag="sc", name="var")
            nc.vector.scalar_tensor_tensor(
                out=var[:], in0=s2ps[:], scalar=c, in1=mu2[:],
                op0=ALU.mult, op1=ALU.subtract)
            lnv = pst.tile([128, T], F32, tag="sc", name="lnv")
            nc.scalar.activation(out=lnv[:], in_=var[:], func=AF.Ln, bias=eps_col[:])
            nc.vector.tensor_scalar_mul(nmu_rep[:], mu[:], -1.0)
            nc.scalar.activation(out=A_rep[:], in_=lnv[:], func=AF.Exp, scale=-0.5)

        # ---------- S1b: x_hat = (x - mu) * rstd ----------
        with tc.tile_pool(name="xh_t", bufs=2) as pxt:
            for dn in range(8):
                t = pxt.tile([128, T], BF16, tag="xh_t")
                nc.vector.tensor_tensor(out=t[:], in0=xT[dn][:], in1=nmu_rep[:],
                                        op=ALU.add)
                nc.vector.tensor_tensor(out=xh[dn][:], in0=t[:], in1=A_rep[:],
                                        op=ALU.mult)
        ln1_cm.__exit__(None, None, None)
        xTp_cm.__exit__(None, None, None)

        # v-half of wqkv loads while q/k project
        for dn in range(8):
            nc.gpsimd.dma_start(out=wv_sb[dn][:], in_=wqkv[dn * 128:(dn + 1) * 128, 2 * D:3 * D])

        # ---------- S1c: q/k projections + rope ----------
        qT_cm = tc.tile_pool(name="qT", bufs=8, side="right")
        qTp = qT_cm.__enter__()
        kT_cm = tc.tile_pool(name="kT", bufs=8, side="right")
        kTp = kT_cm.__enter__()
        qT = [qTp.tile([128, MY], BF16, tag="qT", name=f"qT{i}") for i in range(8)]
        kT = [kTp.tile([128, T], BF16, tag="kT", name=f"kT{i}") for i in range(8)]

        def rope_tile(pool, ps, col0, dst, bias_col):
            """ps: [128, MY] PSUM fp32 (pre-rope q/k feature block, 2 heads).
            dst: [128, MY] bf16 SBUF slice. out = rope(ps)*A (A folded in cos
            tables already? no: A applied via x_hat upstream) + bias."""
            qk = pool.tile([128, MY], BF16, tag="qk", name="qk")
            nc.scalar.activation(out=qk[:], in_=ps[:], func=AF.Copy)
            sw = pool.tile([128, MY], BF16, tag="sw", name="sw")
            for blk in range(4):
                sb = blk ^ 1
                nc.gpsimd.dma_start(
                    out=sw[blk * 32:(blk + 1) * 32, :],
                    in_=qk[sb * 32:(sb + 1) * 32, :])
            cs = slice(col0, col0 + MY)
            t1 = pool.tile([128, MY], BF16, tag="t1", name="t1")
            nc.vector.tensor_tensor(out=t1[:], in0=qk[:], in1=cos_sb[:, cs],
                                    op=ALU.mult)
            t2 = pool.tile([128, MY], BF16, tag="t2", name="t2")
            nc.vector.tensor_tensor(out=t2[:], in0=sw[:], in1=sinsw_sb[:, cs],
                                    op=ALU.mult)
            nc.vector.scalar_tensor_tensor(
                out=dst, in0=t1[:], scalar=bias_col, in1=t2[:],
                op0=ALU.add, op1=ALU.add)

        with tc.tile_pool(name="phQK", bufs=3, side="right") as pqk, \
             tc.tile_pool(name="phQK_ps", bufs=4, space="PSUM") as pqk_ps:
            for kind in range(2):  # 0=q, 1=k
                nhalf = 1 if kind == 0 else 2
                for ft in range(8):
                    for hf in range(nhalf):
                        col0 = hf * MY
                        ps = pqk_ps.tile([128, MY], F32, tag="qk_ps")
                        for dn in range(8):
                            for ns in range(2):
                                nc.tensor.matmul(
                                    ps[:, ns * 512:(ns + 1) * 512],
                                    wqk_sb[dn][:, kind * D + ft * 128:kind * D + (ft + 1) * 128],
                                    xh[dn][:, col0 + ns * 512:col0 + (ns + 1) * 512],
                                    start=(dn == 0), stop=(dn == 7))
                        dst = qT[ft][:] if kind == 0 else kT[ft][:, col0:col0 + MY]
                        rope_tile(pqk, ps, col0, dst, bqc[:, kind * 8 + ft:kind * 8 + ft + 1])

        wqk_cm.__exit__(None, None, None)

        # ---------- S1d: v projection (token-major, with ones column) ----------
        va_cm = tc.tile_pool(name="va", bufs=16, side="right")
        vap = va_cm.__enter__()
        v_aug = [vap.tile([128, H * (HD + 1)], BF16, tag="va", name=f"va{i}") for i in range(KC)]
        with tc.tile_pool(name="phV_ps", bufs=4, space="PSUM") as pv_ps:
            for tt in range(KC):
                ps = pv_ps.tile([128, D], F32, tag="v_ps")
                for dn in range(8):
                    for ns in range(2):
                        nc.tensor.matmul(
                            ps[:, ns * 512:(ns + 1) * 512],
                            xh[dn][:, tt * 128:(tt + 1) * 128],
                            wv_sb[dn][:, ns * 512:(ns + 1) * 512],
                            start=(dn == 0),
                            stop=(not has_vbias and dn == 7))
                if has_vbias:
                    for ns in range(2):
                        nc.tensor.matmul(
                            ps[:, ns * 512:(ns + 1) * 512],
                            onesrow[:, tt * 128:(tt + 1) * 128],
                            bqv[:, ns * 512:(ns + 1) * 512],
                            start=False, stop=True)
                va = v_aug[tt]
                va_v = va[:].rearrange("p (h c) -> p h c", c=HD + 1)
                ps_v = ps[:].rearrange("p (h c) -> p h c", c=HD)
                nc.scalar.activation(out=va_v[:, :, 0:HD], in_=ps_v[:, :, :],
                                     func=AF.Copy)
                nc.vector.memset(va_v[:, :, HD:HD + 1], 1.0)

        wv_cm.__exit__(None, None, None)
        xh_cm.__exit__(None, None, None)
        cossin_cm.__exit__(None, None, None)

        # x_new pool first on the left stack: it outlives the Wo group
        xn_cm = tc.tile_pool(name="xn", bufs=8)
        xnp = xn_cm.__enter__()
        x_new = [xnp.tile([128, MY], F32, tag="xn", name=f"xn{i}") for i in range(8)]

        # wo prefetch (small); xT_own later during attention
        wo_cm = tc.tile_pool(name="wo", bufs=8)
        wop = wo_cm.__enter__()
        wo_sb = [wop.tile([128, D], BF16, tag="wo", name=f"wo{i}") for i in range(8)]
        for dn in range(8):
            nc.gpsimd.dma_start(out=wo_sb[dn][:], in_=wo[dn * 128:(dn + 1) * 128, :])

        # ---------- S2: attention ----------
        attn_cm = tc.tile_pool(name="attnT", bufs=8)
        attnp = attn_cm.__enter__()
        attnT = [attnp.tile([128, MY], BF16, tag="attnT", name=f"attnT{i}") for i in range(8)]

        xo_cm = tc.tile_pool(name="xo", bufs=8)
        xop = xo_cm.__enter__()
        xo = [xop.tile([128, MY], F32, tag="xo", name=f"xo{i}") for i in range(8)]

        with tc.tile_pool(name="phA_exp", bufs=4) as pex, \
             tc.tile_pool(name="phA_t", bufs=2) as pat, \
             tc.tile_pool(name="phA_t1", bufs=1) as pat1, \
             tc.tile_pool(name="phA_s_ps", bufs=2, space="PSUM") as ps_s, \
             tc.tile_pool(name="phA_pv_ps", bufs=2, space="PSUM") as ps_pv:
            for h in range(H):
                hp, par = h // 2, h % 2
                rs = slice(par * 64, (par + 1) * 64)
                pv = ps_pv.tile([HD + 1, MY], F32, tag="pv_ps", name="pv_ps")
                for c in range(KC):
                    ps = ps_s.tile([128, MY], F32, tag="s_ps", name="s_ps")
                    for ns in range(MY // 512):
                        nc.tensor.matmul(
                            ps[:, ns * 512:(ns + 1) * 512],
                            kT[hp][rs, c * 128:(c + 1) * 128],
                            qT[hp][rs, ns * 512:(ns + 1) * 512],
                            start=True, stop=True)
                    ex = pex.tile([128, MY], BF16, tag="exp", name="exp")
                    nc.scalar.activation(out=ex[:], in_=ps[:], func=AF.Exp, scale=SCALE)
                    for ns in range(MY // 512):
                        nc.tensor.matmul(
                            pv[:, ns * 512:(ns + 1) * 512],
                            v_aug[c][:, h * (HD + 1):(h + 1) * (HD + 1)],
                            ex[:, ns * 512:(ns + 1) * 512],
                            start=(c == 0), stop=(c == KC - 1))
                rb = pat.tile([64, MY], F32, tag="rb", name="rb")
                if h == H - 1:
                    # free the PSUM fast on the last head so Wo can start
                    pv_sb = pat1.tile([HD + 1, MY], F32, tag="pv_sb", name="pv_sb")
                    nc.vector.tensor_copy(pv_sb[:], pv[:])
                    src_pv = pv_sb
                else:
                    src_pv = pv
                nc.vector.reciprocal(out=rb[0:1, :], in_=src_pv[HD:HD + 1, :])
                for step in (1, 2, 4, 8, 16, 32):
                    nc.gpsimd.dma_start(out=rb[step:2 * step, :],
                                        in_=rb[0:step, :])
                nc.vector.tensor_tensor(
                    out=attnT[hp][rs, :], in0=src_pv[0:HD, :], in1=rb[:],
                    op=ALU.mult)
                if h == 6:
                    # late prefetch: residual input, hidden under attention
                    for dn in range(8):
                        nc.gpsimd.dma_start(
                            out=xo[dn][:], in_=xT_own[dn * 128:(dn + 1) * 128, :])

        va_cm.__exit__(None, None, None)
        kT_cm.__exit__(None, None, None)
        qT_cm.__exit__(None, None, None)

        # ---------- S2b: Wo + residual -> x_newT (feature-major) ----------
        with tc.tile_pool(name="phWo_ps", bufs=2, space="PSUM") as po_ps:
            for do in range(8):
                ps = po_ps.tile([128, MY], F32, tag="wo_ps")
                for hp in range(8):
                    for ns in range(2):
                        nc.tensor.matmul(
                            ps[:, ns * 512:(ns + 1) * 512],
                            wo_sb[hp][:, do * 128:(do + 1) * 128],
                            attnT[hp][:, ns * 512:(ns + 1) * 512],
                            start=(hp == 0), stop=(hp == 7))
                nc.vector.scalar_tensor_tensor(
                    out=x_new[do][:], in0=ps[:], scalar=boc[:, do:do + 1],
                    in1=xo[do][:], op0=ALU.add, op1=ALU.add)

        xo_cm.__exit__(None, None, None)
        attn_cm.__exit__(None, None, None)
        wo_cm.__exit__(None, None, None)

        # w1 loads into freed attention space; hides under LN2
        w1_cm = tc.tile_pool(name="w1", bufs=8)
        w1p = w1_cm.__enter__()
        w1_sb = [w1p.tile([128, FF], BF16, tag="w1", name=f"w1{i}") for i in range(8)]
        for dn in range(8):
            nc.gpsimd.dma_start(out=w1_sb[dn][:], in_=w1[dn * 128:(dn + 1) * 128, :])

        # ---------- S2c: LN2 -> x_hat2 ----------
        xh2_cm = tc.tile_pool(name="xh2", bufs=8)
        xh2p = xh2_cm.__enter__()
        xh2 = [xh2p.tile([128, MY], BF16, tag="xh2", name=f"xh2{i}") for i in range(8)]
        with tc.tile_pool(name="ln2", bufs=2) as pl2, \
             tc.tile_pool(name="ln2b", bufs=1) as pl2b, \
             tc.tile_pool(name="ln2_ps", bufs=1, space="PSUM") as pl2_ps:
            s1ps = pl2_ps.tile([128, MY], F32, tag="l2s1", name="l2s1")
            s2ps = pl2_ps.tile([128, MY], F32, tag="l2s2", name="l2s2")
            for dn in range(8):
                xnb = pl2.tile([128, MY], BF16, tag="l2xb")
                nc.vector.tensor_copy(xnb[:], x_new[dn][:])
                x2 = pl2.tile([128, MY], BF16, tag="l2x2")
                nc.scalar.activation(out=x2[:], in_=xnb[:], func=AF.Square)
                for j in range(2):
                    nc.tensor.matmul(
                        s1ps[:, j * 512:(j + 1) * 512], ones_bf[:],
                        xnb[:, j * 512:(j + 1) * 512],
                        start=(dn == 0), stop=(dn == 7))
                    nc.tensor.matmul(
                        s2ps[:, j * 512:(j + 1) * 512], ones_bf[:],
                        x2[:, j * 512:(j + 1) * 512],
                        start=(dn == 0), stop=(dn == 7))
            c = 1.0 / D
            mu = pl2.tile([128, MY], F32, tag="l2sc", name="l2mu")
            nc.vector.tensor_scalar_mul(mu[:], s1ps[:], c)
            nmu2_rep = pl2b.tile([128, MY], F32, name="nmu2_rep")
            nc.vector.tensor_scalar_mul(nmu2_rep[:], mu[:], -1.0)
            mu2 = pl2.tile([128, MY], F32, tag="l2sc", name="l2mu2")
            nc.vector.tensor_tensor(out=mu2[:], in0=mu[:], in1=mu[:], op=ALU.mult)
            var = pl2.tile([128, MY], F32, tag="l2sc", name="l2var")
            nc.vector.scalar_tensor_tensor(
                out=var[:], in0=s2ps[:], scalar=c, in1=mu2[:],
                op0=ALU.mult, op1=ALU.subtract)
            lnv = pl2.tile([128, MY], F32, tag="l2sc", name="l2lnv")
            nc.scalar.activation(out=lnv[:], in_=var[:], func=AF.Ln, bias=eps_col[:])
            A2_rep = pl2b.tile([128, MY], F32, name="A2_rep")
            nc.scalar.activation(out=A2_rep[:], in_=lnv[:], func=AF.Exp, scale=-0.5)
            for dn in range(8):
                t = pl2.tile([128, MY], F32, tag="l2t")
                nc.vector.tensor_tensor(out=t[:], in0=x_new[dn][:], in1=nmu2_rep[:],
                                        op=ALU.add)
                nc.vector.tensor_tensor(out=xh2[dn][:], in0=t[:], in1=A2_rep[:],
                                        op=ALU.mult)

        # ---------- S3: MLP ----------
        g1_cm = tc.tile_pool(name="g1T", bufs=32, side="right")
        g1p = g1_cm.__enter__()
        g1T = [g1p.tile([128, MY], BF16, tag="g1T", name=f"g1T{i}") for i in range(32)]
        w2_cm = tc.tile_pool(name="w2d", bufs=3, side="right")
        w2pp = w2_cm.__enter__()
        w2d = [w2pp.tile([128, FF], BF16, tag="w2d", name=f"w2d{i}") for i in range(3)]
        nc.gpsimd.dma_start(out=w2d[0][:], in_=w2p[0:128, :])

        with tc.tile_pool(name="phF1_ps", bufs=3, space="PSUM") as pf1_ps:
            for fc in range(32):
                ps = pf1_ps.tile([128, MY], F32, tag="g1_ps")
                for dn in range(8):
                    for ns in range(2):
                        nc.tensor.matmul(
                            ps[:, ns * 512:(ns + 1) * 512],
                            w1_sb[dn][:, fc * 128:(fc + 1) * 128],
                            xh2[dn][:, ns * 512:(ns + 1) * 512],
                            start=(dn == 0), stop=(dn == 7))
                nc.scalar.activation(out=g1T[fc][:], in_=ps[:], func=AF.Gelu,
                                     bias=b1c[:, fc:fc + 1])

        xh2_cm.__exit__(None, None, None)
        w1_cm.__exit__(None, None, None)

        with tc.tile_pool(name="phF2", bufs=2) as pf2, \
             tc.tile_pool(name="phF2_ps", bufs=2, space="PSUM") as pf2_ps:
            for do in range(8):
                if do + 1 < 8:
                    nc.gpsimd.dma_start(
                        out=w2d[(do + 1) % 3][:],
                        in_=w2p[(do + 1) * 128:(do + 2) * 128, :])
                ps = pf2_ps.tile([128, MY], F32, tag="m_ps")
                for fc in range(32):
                    for ns in range(2):
                        nc.tensor.matmul(
                            ps[:, ns * 512:(ns + 1) * 512],
                            w2d[do % 3][:, fc * 128:(fc + 1) * 128],
                            g1T[fc][:, ns * 512:(ns + 1) * 512],
                            start=(fc == 0), stop=(fc == 31))
                ot = pf2.tile([128, MY], F32, tag="out_t")
                nc.vector.scalar_tensor_tensor(
                    out=ot[:], in0=ps[:], scalar=b2c[:, do:do + 1],
                    in1=x_new[do][:], op0=ALU.add, op1=ALU.add)
                nc.gpsimd.dma_start(out=outT[do * 128:(do + 1) * 128, :], in_=ot[:])

        w2_cm.__exit__(None, None, None)
        g1_cm.__exit__(None, None, None)
        xn_cm.__exit__(None, None, None)
        consts_cm.__exit__(None, None, None)

    split_multi_waits(nc)
    dedupe_ldweights(nc)
    return nc


_PROG_CACHE = {}


def _get_program(has_vbias):
    if has_vbias not in _PROG_CACHE:
        _PROG_CACHE[has_vbias] = build_program(has_vbias)
    return _PROG_CACHE[has_vbias]


def kernel(x, rope_cos, rope_sin, ln1_g, ln1_b, Wqkv, bqkv, Wo, bo, ln2_g, ln2_b,
           W1, b1, W2, b2):
    x = np.asarray(x, np.float32)
    rope_cos = np.asarray(rope_cos, np.float32)
    rope_sin = np.asarray(rope_sin, np.float32)
    Wqkv = np.asarray(Wqkv, np.float32); Wo = np.asarray(Wo, np.float32)
    W1 = np.asarray(W1, np.float32); W2 = np.asarray(W2, np.float32)
    ln1_g = np.asarray(ln1_g, np.float32); ln1_b = np.asarray(ln1_b, np.float32)
    ln2_g = np.asarray(ln2_g, np.float32); ln2_b = np.asarray(ln2_b, np.float32)
    bqkv = np.asarray(bqkv, np.float32); bo = np.asarray(bo, np.float32)
    b1 = np.asarray(b1, np.float32); b2 = np.asarray(b2, np.float32)

    # fold LN gains into weights; LN biases into projection biases
    Wqkv_f = ln1_g[:, None] * Wqkv
    bq_eff = bqkv + ln1_b @ Wqkv          # [3072]
    nc = _get_program(bool(np.any(bq_eff[2 * D:] != 0)))
    W1_f = ln2_g[:, None] * W1
    b1_eff = b1 + ln2_b @ W1              # [4096]

    wqkv_bf = np.ascontiguousarray(Wqkv_f.astype(BF))
    wo_bf = np.ascontiguousarray(Wo.astype(BF))
    w1_bf = np.ascontiguousarray(W1_f.astype(BF))
    w2p = np.ascontiguousarray(
        W2.reshape(32, 128, 8, 128).transpose(2, 1, 0, 3).reshape(D, FF).astype(BF))

    # q/k bias columns [128, 16]: col kind*8+ft = bq_eff[kind*D+ft*128 : +128]
    bqc = np.ascontiguousarray(
        bq_eff[:2 * D].reshape(16, 128).T.astype(np.float32))
    bqv = np.ascontiguousarray(bq_eff[None, 2 * D:].astype(BF))
    b1c = np.ascontiguousarray(b1_eff.reshape(FF // 128, 128).T.astype(np.float32))
    boc = np.ascontiguousarray(bo.reshape(8, 128).T.astype(np.float32))
    b2c = np.ascontiguousarray(b2.reshape(8, 128).T.astype(np.float32))

    cosT = rope_cos.T  # [32, T]
    sinT = rope_sin.T
    cos_rep = np.tile(cosT, (4, 1))
    sinsw_rep = np.concatenate([-sinT, sinT, -sinT, sinT], 0)

    in_maps = []
    for c in range(8):
        b, h2 = c // 2, c % 2
        perm = np.r_[h2 * MY:(h2 + 1) * MY, (1 - h2) * MY:(2 - h2) * MY]
        xp = x[b][perm]                        # [T, D], own tokens first
        m = {
            "xT_bf": np.ascontiguousarray(xp.T.astype(BF)),
            "xT_own": np.ascontiguousarray(xp[:MY].T),
            "wqkv": wqkv_bf, "wo": wo_bf, "w1": w1_bf, "w2p": w2p,
            "cos_rep": np.ascontiguousarray(cos_rep[:, perm].astype(BF)),
            "sinsw_rep": np.ascontiguousarray(sinsw_rep[:, perm].astype(BF)),
            "bqc": bqc, "bqv": bqv, "b1c": b1c, "boc": boc, "b2c": b2c,
        }
        in_maps.append(m)

    from concourse.bass_utils import run_bass_kernel_spmd
    res = run_bass_kernel_spmd(nc, in_maps, list(range(8)))

    out = np.empty((B, T, D), np.float32)
    for c in range(8):
        b, h2 = c // 2, c % 2
        out[b, h2 * MY:(h2 + 1) * MY, :] = res.results[c]["outT"].T
    return out



# revision 1
# speedup vs baseline: 1.2054x; 1.2054x over previous
"""Trainium2 Bass kernel for a dense transformer layer (B=4, T=2048, D=1024,
H=16, HD=64, FF=4096, fp32 I/O).

Sharding (8 cores, no cross-core communication): core c handles batch c//2 and
token-half c%2; per-core inputs are permuted so the core's own 1024 tokens come
first. K/V cover all 2048 tokens of the batch (2x redundant); Q/attention/Wo/MLP
cover only the core's 1024 tokens.

v1 redesign vs the token-major baseline:
- Feature-major activations end to end: x arrives transposed from the host, the
  residual stream stays feature-major, and the output is written transposed
  (host un-transposes). This removes all PE transposes (and enables walrus'
  redundant-LDWEIGHTS optimization, which is incompatible with transpose-mode
  weight loads).
- LayerNorm stats via ones-vector matmuls over feature chunks (PSUM row
  accumulation), normalization applied with gpsimd partition_broadcast'ed
  mean/rstd rows; LN gains/biases and all projection biases are folded into
  weights host-side, rank-1 corrections ride as K=1 matmuls.
- rstd = exp(-0.5*log(var+eps)) keeps ACT in the natural_log_exp table set
  (shared with attention's exp), avoiding Sqrt table switches.
- Rope partner-swap via 4 SBUF->SBUF DMAs per tile instead of 4 narrow DVE
  tensor_tensor ops; softmax denominators via reciprocal_approx_fast (~5x
  faster than reciprocal) broadcast on gpsimd.
"""

import sys

sys.path.insert(0, "/opt/trn_rl_repo")

import dataclasses

import numpy as np
import ml_dtypes

import concourse.bass as bass
import concourse.tile as tile
from concourse import library_config, mybir
from concourse.vector_clock import ScopedClock

F32 = mybir.dt.float32
BF16 = mybir.dt.bfloat16
AF = mybir.ActivationFunctionType
ALU = mybir.AluOpType

B, T, D = 4, 2048, 1024
H, HD = 16, 64
FF = 4 * D
MY = T // 2          # tokens owned by one core
KC = T // 128        # k chunks of 128 tokens
EPS = 1e-5
SCALE = 1.0 / 8.0    # 1/sqrt(HD)

BF = ml_dtypes.bfloat16


class PatchedTileContext(tile.TileContext):
    """walrus in this container accepts a single sync-wait per instruction;
    split the tail drain's waits across chained drains."""

    def _drain_and_barrier(self, tick_clock, wait_clock):
        drain_inst = self.nc.sync.drain()
        wait_clock.add_sem_waits(
            drain_inst.ins, ScopedClock({None: tick_clock.global_clock})
        )
        si = drain_inst.ins.sync_info
        waits = list(si.on_wait) if si and si.on_wait else []
        if len(waits) > 1:
            si.on_wait = waits[:1]
            for w in waits[1:]:
                d2 = self.nc.sync.drain()
                si2 = d2.ins.sync_info
                if si2 is None:
                    d2.ins.sync_info = mybir.SyncInfo(on_wait=[w], on_update=[])
                else:
                    si2.on_wait = [w]
        self.nc.all_engine_barrier()
        assert self.sems is not None
        popped = self.nc._tile_sem_poison_stack.pop()
        assert popped is self._sem_poison
        self.nc.clear_and_free_semaphores(list(self.sems.allocated().values()))
        self.nc.all_engine_barrier()


def split_multi_waits(nc, max_waits=1):
    """Move extra sync-waits onto NoOps inserted just before the over-limit
    instruction (same engine, program order preserved)."""
    template = nc.vector.nop().ins
    fn = nc.m.functions[0]
    ctr = 0
    for bb in fn.blocks:
        out = []
        for inst in bb.instructions:
            si = getattr(inst, "sync_info", None)
            waits = list(si.on_wait) if (si and si.on_wait) else []
            if len(waits) > max_waits:
                for w in waits[:-max_waits]:
                    ctr += 1
                    nop = dataclasses.replace(
                        template,
                        name=f"IWS-{ctr}",
                        engine=inst.engine,
                        ins=[],
                        outs=[],
                        sync_info=mybir.SyncInfo(on_wait=[w], on_update=[]),
                    )
                    nc.register_instruction(nop, overwrite=True)
                    out.append(nop)
                si.on_wait = waits[-max_waits:]
            out.append(inst)
        bb.instructions[:] = out
    return ctr


def dedupe_ldweights(nc):
    """Remove InstLdweights whose stationary operand is identical to the
    immediately preceding weight load, with only PE matmuls/noops in between
    (weights persist in the PE array across matmuls). Conservative: keeps
    any LDW carrying sem waits, and resets tracking whenever a non-PE
    instruction appears (it may rewrite the SBUF weight region)."""
    fn = nc.m.functions[0]
    removed = 0
    for bb in fn.blocks:
        out = []
        prev_key = None
        for inst in bb.instructions:
            if isinstance(inst, mybir.InstLdweights):
                ap = inst.ins[0]
                key = (ap.memref, ap.offset, str(ap.ap), str(ap.dtype),
                       getattr(inst, "is_transpose", None),
                       getattr(inst, "perf_mode", None),
                       str(getattr(inst, "tile_position", None)))
                si = inst.sync_info
                has_sync = bool(si and (si.on_wait or si.on_update))
                if key == prev_key and not has_sync:
                    removed += 1
                    continue
                prev_key = key
            elif isinstance(inst, (mybir.InstMatmult, mybir.InstNoOp)):
                pass
            else:
                prev_key = None
            out.append(inst)
        bb.instructions[:] = out
    return removed


def build_program(has_vbias=True):
    nc = bass.Bass()

    # ---- I/O (all weights host-folded: W' = ln_g ⊙ W; biases separate) ----
    xT_bf = nc.declare_dram_parameter("xT_bf", [D, T], BF16, isOutput=False)
    xT_own = nc.declare_dram_parameter("xT_own", [D, MY], F32, isOutput=False)
    wqkv = nc.declare_dram_parameter("wqkv", [D, 3 * D], BF16, isOutput=False)
    wo = nc.declare_dram_parameter("wo", [D, D], BF16, isOutput=False)
    w1 = nc.declare_dram_parameter("w1", [D, FF], BF16, isOutput=False)
    # w2p[do*128+p, fc*128+j] = W2[fc*128+p, do*128+j]
    w2p = nc.declare_dram_parameter("w2p", [D, FF], BF16, isOutput=False)
    cos_in = nc.declare_dram_parameter("cos_rep", [128, T], BF16, isOutput=False)
    sinsw_in = nc.declare_dram_parameter("sinsw_rep", [128, T], BF16, isOutput=False)
    bqc_in = nc.declare_dram_parameter("bqc", [128, 16], F32, isOutput=False)  # q/k bias cols
    bqv_in = nc.declare_dram_parameter("bqv", [1, D], BF16, isOutput=False)    # v bias row
    b1c_in = nc.declare_dram_parameter("b1c", [128, FF // 128], F32, isOutput=False)
    boc_in = nc.declare_dram_parameter("boc", [128, 8], F32, isOutput=False)
    b2c_in = nc.declare_dram_parameter("b2c", [128, 8], F32, isOutput=False)
    outT = nc.declare_dram_parameter("outT", [D, MY], F32, isOutput=True)

    with PatchedTileContext(nc) as tc:
        consts_cm = tc.tile_pool(name="consts", bufs=1)
        consts = consts_cm.__enter__()
        ones_bf = consts.tile([128, 128], BF16)
        nc.vector.memset(ones_bf[:], 1.0)
        eps_col = consts.tile([128, 1], F32)
        nc.vector.memset(eps_col[:], EPS)
        onesrow = consts.tile([1, T], BF16)
        nc.vector.memset(onesrow[:], 1.0)
        ones64 = consts.tile([1, 64], BF16)
        nc.vector.memset(ones64[:], 1.0)
        bqc = consts.tile([128, 16], F32)
        nc.gpsimd.dma_start(out=bqc[:], in_=bqc_in[:])
        bqv = consts.tile([1, D], BF16)
        nc.gpsimd.dma_start(out=bqv[:], in_=bqv_in[:])
        b1c = consts.tile([128, FF // 128], F32)
        nc.gpsimd.dma_start(out=b1c[:], in_=b1c_in[:])
        boc = consts.tile([128, 8], F32)
        nc.gpsimd.dma_start(out=boc[:], in_=boc_in[:])
        b2c = consts.tile([128, 8], F32)
        nc.gpsimd.dma_start(out=b2c[:], in_=b2c_in[:])

        cossin_cm = tc.tile_pool(name="cossin", bufs=1)
        cossin = cossin_cm.__enter__()
        cos_sb = cossin.tile([128, T], BF16)
        sinsw_sb = cossin.tile([128, T], BF16)

        # x_hat pool opened early (left stack: outlives wqk/wv/ln1)
        xh_cm = tc.tile_pool(name="xh", bufs=8)
        xhp = xh_cm.__enter__()
        xh = [xhp.tile([128, T], BF16, tag="xh", name=f"xh{i}") for i in range(8)]

        # v-half of wqkv: loaded during q/k projections, dies after v
        wv_cm = tc.tile_pool(name="wv", bufs=8)
        wvp = wv_cm.__enter__()
        wv_sb = [wvp.tile([128, D], BF16, tag="wv", name=f"wv{i}") for i in range(8)]

        wqk_cm = tc.tile_pool(name="wqk", bufs=8)
        wqkp = wqk_cm.__enter__()
        wqk_sb = [wqkp.tile([128, 2 * D], BF16, tag="wqk", name=f"wqk{i}") for i in range(8)]

        # ---------- S1a: load xT (right stack), stats via ones-matmuls ----------
        xTp_cm = tc.tile_pool(name="xT", bufs=8, side="right")
        xTp = xTp_cm.__enter__()
        xT = [xTp.tile([128, T], BF16, tag="xT", name=f"xT{i}") for i in range(8)]
        for dn in range(8):
            nc.gpsimd.dma_start(out=xT[dn][:], in_=xT_bf[dn * 128:(dn + 1) * 128, :])
        nc.gpsimd.dma_start(out=cos_sb[:], in_=cos_in[:])
        nc.gpsimd.dma_start(out=sinsw_sb[:], in_=sinsw_in[:])
        for dn in range(8):
            nc.gpsimd.dma_start(out=wqk_sb[dn][:], in_=wqkv[dn * 128:(dn + 1) * 128, 0:2 * D])

        ln1_cm = tc.tile_pool(name="ln1", bufs=1)
        ln1 = ln1_cm.__enter__()
        A_rep = ln1.tile([128, T], BF16)      # rstd broadcast
        nmu_rep = ln1.tile([128, T], BF16)    # -mu broadcast

        with tc.tile_pool(name="s1stat", bufs=2) as pst, \
             tc.tile_pool(name="s1stat_ps", bufs=1, space="PSUM") as pst_ps:
            s1ps = pst_ps.tile([128, T], F32, tag="s1ps", name="s1ps")
            s2ps = pst_ps.tile([128, T], F32, tag="s2ps", name="s2ps")
            for dn in range(8):
                x2 = pst.tile([128, T], BF16, tag="x2")
                nc.scalar.activation(out=x2[:], in_=xT[dn][:], func=AF.Square)
                for j in range(4):
                    nc.tensor.matmul(
                        s1ps[:, j * 512:(j + 1) * 512], ones_bf[:],
                        xT[dn][:, j * 512:(j + 1) * 512],
                        start=(dn == 0), stop=(dn == 7))
                    nc.tensor.matmul(
                        s2ps[:, j * 512:(j + 1) * 512], ones_bf[:],
                        x2[:, j * 512:(j + 1) * 512],
                        start=(dn == 0), stop=(dn == 7))
            c = 1.0 / D
            mu = pst.tile([128, T], F32, tag="sc", name="mu")
            nc.vector.tensor_scalar_mul(mu[:], s1ps[:], c)
            mu2 = pst.tile([128, T], F32, tag="sc", name="mu2")
            nc.vector.tensor_tensor(out=mu2[:], in0=mu[:], in1=mu[:], op=ALU.mult)
            var = pst.tile([128, T], F32, tag="sc", name="var")
            nc.vector.scalar_tensor_tensor(
                out=var[:], in0=s2ps[:], scalar=c, in1=mu2[:],
                op0=ALU.mult, op1=ALU.subtract)
            lnv = pst.tile([128, T], F32, tag="sc", name="lnv")
            nc.scalar.activation(out=lnv[:], in_=var[:], func=AF.Ln, bias=eps_col[:])
            nc.vector.tensor_scalar_mul(nmu_rep[:], mu[:], -1.0)
            nc.scalar.activation(out=A_rep[:], in_=lnv[:], func=AF.Exp, scale=-0.5)

        # ---------- S1b: x_hat = (x - mu) * rstd ----------
        with tc.tile_pool(name="xh_t", bufs=2) as pxt:
            for dn in range(8):
                t = pxt.tile([128, T], BF16, tag="xh_t")
                nc.vector.tensor_tensor(out=t[:], in0=xT[dn][:], in1=nmu_rep[:],
                                        op=ALU.add)
                nc.vector.tensor_tensor(out=xh[dn][:], in0=t[:], in1=A_rep[:],
                                        op=ALU.mult)
        ln1_cm.__exit__(None, None, None)
        xTp_cm.__exit__(None, None, None)

        # v-half of wqkv loads while q/k project
        for dn in range(8):
            nc.gpsimd.dma_start(out=wv_sb[dn][:], in_=wqkv[dn * 128:(dn + 1) * 128, 2 * D:3 * D])

        # ---------- S1c: q/k projections + rope ----------
        qT_cm = tc.tile_pool(name="qT", bufs=8, side="right")
        qTp = qT_cm.__enter__()
        kT_cm = tc.tile_pool(name="kT", bufs=8, side="right")
        kTp = kT_cm.__enter__()
        qT = [qTp.tile([128, MY], BF16, tag="qT", name=f"qT{i}") for i in range(8)]
        kT = [kTp.tile([128, T], BF16, tag="kT", name=f"kT{i}") for i in range(8)]

        def rope_tile(pool, ps, col0, dst, bias_col):
            """ps: [128, MY] PSUM fp32 (pre-rope q/k feature block, 2 heads).
            dst: [128, MY] bf16 SBUF slice. out = rope(ps)*A (A folded in cos
            tables already? no: A applied via x_hat upstream) + bias."""
            qk = pool.tile([128, MY], BF16, tag="qk", name="qk")
            nc.scalar.activation(out=qk[:], in_=ps[:], func=AF.Copy)
            sw = pool.tile([128, MY], BF16, tag="sw", name="sw")
            for blk in range(4):
                sb = blk ^ 1
                nc.gpsimd.dma_start(
                    out=sw[blk * 32:(blk + 1) * 32, :],
                    in_=qk[sb * 32:(sb + 1) * 32, :])
            cs = slice(col0, col0 + MY)
            t1 = pool.tile([128, MY], BF16, tag="t1", name="t1")
            nc.vector.tensor_tensor(out=t1[:], in0=qk[:], in1=cos_sb[:, cs],
                                    op=ALU.mult)
            t2 = pool.tile([128, MY], BF16, tag="t2", name="t2")
            nc.vector.tensor_tensor(out=t2[:], in0=sw[:], in1=sinsw_sb[:, cs],
                                    op=ALU.mult)
            nc.vector.scalar_tensor_tensor(
                out=dst, in0=t1[:], scalar=bias_col, in1=t2[:],
                op0=ALU.add, op1=ALU.add)

        with tc.tile_pool(name="phQK", bufs=3, side="right") as pqk, \
             tc.tile_pool(name="phQK_ps", bufs=4, space="PSUM") as pqk_ps:
            for kind in range(2):  # 0=q, 1=k
                nhalf = 1 if kind == 0 else 2
                for ft in range(8):
                    for hf in range(nhalf):
                        col0 = hf * MY
                        ps = pqk_ps.tile([128, MY], F32, tag="qk_ps")
                        for dn in range(8):
                            for ns in range(2):
                                nc.tensor.matmul(
                                    ps[:, ns * 512:(ns + 1) * 512],
                                    wqk_sb[dn][:, kind * D + ft * 128:kind * D + (ft + 1) * 128],
                                    xh[dn][:, col0 + ns * 512:col0 + (ns + 1) * 512],
                                    start=(dn == 0), stop=(dn == 7))
                        dst = qT[ft][:] if kind == 0 else kT[ft][:, col0:col0 + MY]
                        rope_tile(pqk, ps, col0, dst, bqc[:, kind * 8 + ft:kind * 8 + ft + 1])

        wqk_cm.__exit__(None, None, None)

        # ---------- S1d: v projection (token-major, with ones column) ----------
        va_cm = tc.tile_pool(name="va", bufs=16, side="right")
        vap = va_cm.__enter__()
        v_aug = [vap.tile([128, H * (HD + 1)], BF16, tag="va", name=f"va{i}") for i in range(KC)]
        with tc.tile_pool(name="phV_ps", bufs=4, space="PSUM") as pv_ps:
            for tt in range(KC):
                ps = pv_ps.tile([128, D], F32, tag="v_ps")
                for dn in range(8):
                    for ns in range(2):
                        nc.tensor.matmul(
                            ps[:, ns * 512:(ns + 1) * 512],
                            xh[dn][:, tt * 128:(tt + 1) * 128],
                            wv_sb[dn][:, ns * 512:(ns + 1) * 512],
                            start=(dn == 0),
                            stop=(not has_vbias and dn == 7))
                if has_vbias:
                    for ns in range(2):
                        nc.tensor.matmul(
                            ps[:, ns * 512:(ns + 1) * 512],
                            onesrow[:, tt * 128:(tt + 1) * 128],
                            bqv[:, ns * 512:(ns + 1) * 512],
                            start=False, stop=True)
                va = v_aug[tt]
                va_v = va[:].rearrange("p (h c) -> p h c", c=HD + 1)
                ps_v = ps[:].rearrange("p (h c) -> p h c", c=HD)
                nc.scalar.activation(out=va_v[:, :, 0:HD], in_=ps_v[:, :, :],
                                     func=AF.Copy)
                nc.vector.memset(va_v[:, :, HD:HD + 1], 1.0)

        wv_cm.__exit__(None, None, None)
        xh_cm.__exit__(None, None, None)
        cossin_cm.__exit__(None, None, None)

        # x_new pool first on the left stack: it outlives the Wo group
        xn_cm = tc.tile_pool(name="xn", bufs=8)
        xnp = xn_cm.__enter__()
        x_new = [xnp.tile([128, MY], F32, tag="xn", name=f"xn{i}") for i in range(8)]

        # wo prefetch (small); xT_own later during attention
        wo_cm = tc.tile_pool(name="wo", bufs=8)
        wop = wo_cm.__enter__()
        wo_sb = [wop.tile([128, D], BF16, tag="wo", name=f"wo{i}") for i in range(8)]
        for dn in range(8):
            nc.gpsimd.dma_start(out=wo_sb[dn][:], in_=wo[dn * 128:(dn + 1) * 128, :])

        # ---------- S2: attention ----------
        attn_cm = tc.tile_pool(name="attnT", bufs=8)
        attnp = attn_cm.__enter__()
        attnT = [attnp.tile([128, MY], BF16, tag="attnT", name=f"attnT{i}") for i in range(8)]

        xo_cm = tc.tile_pool(name="xo", bufs=8)
        xop = xo_cm.__enter__()
        xo = [xop.tile([128, MY], F32, tag="xo", name=f"xo{i}") for i in range(8)]

        with tc.tile_pool(name="phA_exp", bufs=4) as pex, \
             tc.tile_pool(name="phA_t", bufs=2) as pat, \
             tc.tile_pool(name="phA_t1", bufs=1) as pat1, \
             tc.tile_pool(name="phA_s_ps", bufs=2, space="PSUM") as ps_s, \
             tc.tile_pool(name="phA_pv_ps", bufs=2, space="PSUM") as ps_pv:
            for h in range(H):
                hp, par = h // 2, h % 2
                rs = slice(par * 64, (par + 1) * 64)
                pv = ps_pv.tile([HD + 1, MY], F32, tag="pv_ps", name="pv_ps")
                for c in range(KC):
                    ps = ps_s.tile([128, MY], F32, tag="s_ps", name="s_ps")
                    for ns in range(MY // 512):
                        nc.tensor.matmul(
                            ps[:, ns * 512:(ns + 1) * 512],
                            kT[hp][rs, c * 128:(c + 1) * 128],
                            qT[hp][rs, ns * 512:(ns + 1) * 512],
                            start=True, stop=True)
                    ex = pex.tile([128, MY], BF16, tag="exp", name="exp")
                    nc.scalar.activation(out=ex[:], in_=ps[:], func=AF.Exp, scale=SCALE)
                    for ns in range(MY // 512):
                        nc.tensor.matmul(
                            pv[:, ns * 512:(ns + 1) * 512],
                            v_aug[c][:, h * (HD + 1):(h + 1) * (HD + 1)],
                            ex[:, ns * 512:(ns + 1) * 512],
                            start=(c == 0), stop=(c == KC - 1))
                rb = pat.tile([64, MY], F32, tag="rb", name="rb")
                if h == H - 1:
                    # free the PSUM fast on the last head so Wo can start
                    pv_sb = pat1.tile([HD + 1, MY], F32, tag="pv_sb", name="pv_sb")
                    nc.vector.tensor_copy(pv_sb[:], pv[:])
                    src_pv = pv_sb
                else:
                    src_pv = pv
                nc.vector.reciprocal(out=rb[0:1, :], in_=src_pv[HD:HD + 1, :])
                for step in (1, 2, 4, 8, 16, 32):
                    nc.gpsimd.dma_start(out=rb[step:2 * step, :],
                                        in_=rb[0:step, :])
                nc.vector.tensor_tensor(
                    out=attnT[hp][rs, :], in0=src_pv[0:HD, :], in1=rb[:],
                    op=ALU.mult)
                if h == 6:
                    # late prefetch: residual input, hidden under attention
                    for dn in range(8):
                        nc.gpsimd.dma_start(
                            out=xo[dn][:], in_=xT_own[dn * 128:(dn + 1) * 128, :])

        va_cm.__exit__(None, None, None)
        kT_cm.__exit__(None, None, None)
        qT_cm.__exit__(None, None, None)

        # ---------- S2b: Wo + residual -> x_newT (feature-major) ----------
        with tc.tile_pool(name="phWo_ps", bufs=2, space="PSUM") as po_ps:
            for do in range(8):
                ps = po_ps.tile([128, MY], F32, tag="wo_ps")
                for hp in range(8):
                    for ns in range(2):
                        nc.tensor.matmul(
                            ps[:, ns * 512:(ns + 1) * 512],
                            wo_sb[hp][:, do * 128:(do + 1) * 128],
                            attnT[hp][:, ns * 512:(ns + 1) * 512],
                            start=(hp == 0), stop=(hp == 7))
                nc.vector.scalar_tensor_tensor(
                    out=x_new[do][:], in0=ps[:], scalar=boc[:, do:do + 1],
                    in1=xo[do][:], op0=ALU.add, op1=ALU.add)

        xo_cm.__exit__(None, None, None)
        attn_cm.__exit__(None, None, None)
        wo_cm.__exit__(None, None, None)

        # w1 loads into freed attention space; hides under LN2
        w1_cm = tc.tile_pool(name="w1", bufs=8)
        w1p = w1_cm.__enter__()
        w1_sb = [w1p.tile([128, FF], BF16, tag="w1", name=f"w1{i}") for i in range(8)]
        for dn in range(8):
            nc.gpsimd.dma_start(out=w1_sb[dn][:], in_=w1[dn * 128:(dn + 1) * 128, :])

        # ---------- S2c: LN2 -> x_hat2 ----------
        xh2_cm = tc.tile_pool(name="xh2", bufs=8)
        xh2p = xh2_cm.__enter__()
        xh2 = [xh2p.tile([128, MY], BF16, tag="xh2", name=f"xh2{i}") for i in range(8)]
        with tc.tile_pool(name="ln2", bufs=2) as pl2, \
             tc.tile_pool(name="ln2b", bufs=1) as pl2b, \
             tc.tile_pool(name="ln2_ps", bufs=1, space="PSUM") as pl2_ps:
            s1ps = pl2_ps.tile([128, MY], F32, tag="l2s1", name="l2s1")
            s2ps = pl2_ps.tile([128, MY], F32, tag="l2s2", name="l2s2")
            for dn in range(8):
                xnb = pl2.tile([128, MY], BF16, tag="l2xb")
                nc.vector.tensor_copy(xnb[:], x_new[dn][:])
                x2 = pl2.tile([128, MY], BF16, tag="l2x2")
                nc.scalar.activation(out=x2[:], in_=xnb[:], func=AF.Square)
                for j in range(2):
                    nc.tensor.matmul(
                        s1ps[:, j * 512:(j + 1) * 512], ones_bf[:],
                        xnb[:, j * 512:(j + 1) * 512],
                        start=(dn == 0), stop=(dn == 7))
                    nc.tensor.matmul(
                        s2ps[:, j * 512:(j + 1) * 512], ones_bf[:],
                        x2[:, j * 512:(j + 1) * 512],
                        start=(dn == 0), stop=(dn == 7))
            c = 1.0 / D
            mu = pl2.tile([128, MY], F32, tag="l2sc", name="l2mu")
            nc.vector.tensor_scalar_mul(mu[:], s1ps[:], c)
            nmu2_rep = pl2b.tile([128, MY], F32, name="nmu2_rep")
            nc.vector.tensor_scalar_mul(nmu2_rep[:], mu[:], -1.0)
            mu2 = pl2.tile([128, MY], F32, tag="l2sc", name="l2mu2")
            nc.vector.tensor_tensor(out=mu2[:], in0=mu[:], in1=mu[:], op=ALU.mult)
            var = pl2.tile([128, MY], F32, tag="l2sc", name="l2var")
            nc.vector.scalar_tensor_tensor(
                out=var[:], in0=s2ps[:], scalar=c, in1=mu2[:],
                op0=ALU.mult, op1=ALU.subtract)
            lnv = pl2.tile([128, MY], F32, tag="l2sc", name="l2lnv")
            nc.scalar.activation(out=lnv[:], in_=var[:], func=AF.Ln, bias=eps_col[:])
            A2_rep = pl2b.tile([128, MY], F32, name="A2_rep")
            nc.scalar.activation(out=A2_rep[:], in_=lnv[:], func=AF.Exp, scale=-0.5)
            for dn in range(8):
                t = pl2.tile([128, MY], F32, tag="l2t")
                nc.vector.tensor_tensor(out=t[:], in0=x_new[dn][:], in1=nmu2_rep[:],
                                        op=ALU.add)
                nc.vector.tensor_tensor(out=xh2[dn][:], in0=t[:], in1=A2_rep[:],
                                        op=ALU.mult)

        # ---------- S3: MLP ----------
        g1_cm = tc.tile_pool(name="g1T", bufs=32, side="right")
        g1p = g1_cm.__enter__()
        g1T = [g1p.tile([128, MY], BF16, tag="g1T", name=f"g1T{i}") for i in range(32)]
        w2_cm = tc.tile_pool(name="w2d", bufs=3, side="right")
        w2pp = w2_cm.__enter__()
        w2d = [w2pp.tile([128, FF], BF16, tag="w2d", name=f"w2d{i}") for i in range(3)]
        nc.gpsimd.dma_start(out=w2d[0][:], in_=w2p[0:128, :])

        with tc.tile_pool(name="phF1_ps", bufs=3, space="PSUM") as pf1_ps:
            for fc in range(32):
                ps = pf1_ps.tile([128, MY], F32, tag="g1_ps")
                for dn in range(8):
                    for ns in range(2):
                        nc.tensor.matmul(
                            ps[:, ns * 512:(ns + 1) * 512],
                            w1_sb[dn][:, fc * 128:(fc + 1) * 128],
                            xh2[dn][:, ns * 512:(ns + 1) * 512],
                            start=(dn == 0), stop=(dn == 7))
                nc.scalar.activation(out=g1T[fc][:], in_=ps[:], func=AF.Gelu,
                                     bias=b1c[:, fc:fc + 1])

        xh2_cm.__exit__(None, None, None)
        w1_cm.__exit__(None, None, None)

        with tc.tile_pool(name="phF2", bufs=2) as pf2, \
             tc.tile_pool(name="phF2_ps", bufs=2, space="PSUM") as pf2_ps:
            for do in range(8):
                if do + 1 < 8:
                    nc.gpsimd.dma_start(
                        out=w2d[(do + 1) % 3][:],
                        in_=w2p[(do + 1) * 128:(do + 2) * 128, :])
                ps = pf2_ps.tile([128, MY], F32, tag="m_ps")
                for fc in range(32):
                    for ns in range(2):
                        nc.tensor.matmul(
                            ps[:, ns * 512:(ns + 1) * 512],
                            w2d[do % 3][:, fc * 128:(fc + 1) * 128],
                            g1T[fc][:, ns * 512:(ns + 1) * 512],
                            start=(fc == 0), stop=(fc == 31))
                ot = pf2.tile([128, MY], F32, tag="out_t")
                nc.vector.scalar_tensor_tensor(
                    out=ot[:], in0=ps[:], scalar=b2c[:, do:do + 1],
                    in1=x_new[do][:], op0=ALU.add, op1=ALU.add)
                nc.gpsimd.dma_start(out=outT[do * 128:(do + 1) * 128, :], in_=ot[:])

        w2_cm.__exit__(None, None, None)
        g1_cm.__exit__(None, None, None)
        xn_cm.__exit__(None, None, None)
        consts_cm.__exit__(None, None, None)

    split_multi_waits(nc)
    dedupe_ldweights(nc)
    return nc


_PROG_CACHE = {}


def _get_program(has_vbias):
    if has_vbias not in _PROG_CACHE:
        _PROG_CACHE[has_vbias] = build_program(has_vbias)
    return _PROG_CACHE[has_vbias]


def kernel(x, rope_cos, rope_sin, ln1_g, ln1_b, Wqkv, bqkv, Wo, bo, ln2_g, ln2_b,
           W1, b1, W2, b2):
    x = np.asarray(x, np.float32)
    rope_cos = np.asarray(rope_cos, np.float32)
    rope_sin = np.asarray(rope_sin, np.float32)
    Wqkv = np.asarray(Wqkv, np.float32); Wo = np.asarray(Wo, np.float32)
    W1 = np.asarray(W1, np.float32); W2 = np.asarray(W2, np.float32)
    ln1_g = np.asarray(ln1_g, np.float32); ln1_b = np.asarray(ln1_b, np.float32)
    ln2_g = np.asarray(ln2_g, np.float32); ln2_b = np.asarray(ln2_b, np.float32)
    bqkv = np.asarray(bqkv, np.float32); bo = np.asarray(bo, np.float32)
    b1 = np.asarray(b1, np.float32); b2 = np.asarray(b2, np.float32)

    # fold LN gains into weights; LN biases into projection biases
    Wqkv_f = ln1_g[:, None] * Wqkv
    bq_eff = bqkv + ln1_b @ Wqkv          # [3072]
    nc = _get_program(bool(np.any(bq_eff[2 * D:] != 0)))
    W1_f = ln2_g[:, None] * W1
    b1_eff = b1 + ln2_b @ W1              # [4096]

    wqkv_bf = np.ascontiguousarray(Wqkv_f.astype(BF))
    wo_bf = np.ascontiguousarray(Wo.astype(BF))
    w1_bf = np.ascontiguousarray(W1_f.astype(BF))
    w2p = np.ascontiguousarray(
        W2.reshape(32, 128, 8, 128).transpose(2, 1, 0, 3).reshape(D, FF).astype(BF))

    # q/k bias columns [128, 16]: col kind*8+ft = bq_eff[kind*D+ft*128 : +128]
    bqc = np.ascontiguousarray(
        bq_eff[:2 * D].reshape(16, 128).T.astype(np.float32))
    bqv = np.ascontiguousarray(bq_eff[None, 2 * D:].astype(BF))
    b1c = np.ascontiguousarray(b1_eff.reshape(FF // 128, 128).T.astype(np.float32))
    boc = np.ascontiguousarray(bo.reshape(8, 128).T.astype(np.float32))
    b2c = np.ascontiguousarray(b2.reshape(8, 128).T.astype(np.float32))

    cosT = rope_cos.T  # [32, T]
    sinT = rope_sin.T
    cos_rep = np.tile(cosT, (4, 1))
    sinsw_rep = np.concatenate([-sinT, sinT, -sinT, sinT], 0)

    in_maps = []
    for c in range(8):
        b, h2 = c // 2, c % 2
        perm = np.r_[h2 * MY:(h2 + 1) * MY, (1 - h2) * MY:(2 - h2) * MY]
        xp = x[b][perm]                        # [T, D], own tokens first
        m = {
            "xT_bf": np.ascontiguousarray(xp.T.astype(BF)),
            "xT_own": np.ascontiguousarray(xp[:MY].T),
            "wqkv": wqkv_bf, "wo": wo_bf, "w1": w1_bf, "w2p": w2p,
            "cos_rep": np.ascontiguousarray(cos_rep[:, perm].astype(BF)),
            "sinsw_rep": np.ascontiguousarray(sinsw_rep[:, perm].astype(BF)),
            "bqc": bqc, "bqv": bqv, "b1c": b1c, "boc": boc, "b2c": b2c,
        }
        in_maps.append(m)

    from concourse.bass_utils import run_bass_kernel_spmd
    res = run_bass_kernel_spmd(nc, in_maps, list(range(8)))

    out = np.empty((B, T, D), np.float32)
    for c in range(8):
        b, h2 = c // 2, c % 2
        out[b, h2 * MY:(h2 + 1) * MY, :] = res.results[c]["outT"].T
    return out

